# revision 2
# baseline (speedup 1.0000x reference)
"""GQA attention (B=2, S=2048, D=2048, H=16, KV=4, HD=128) on 8 TRN2 cores.

Sharding: core c -> batch b = c//4, kv-group g = c%4 (4 query heads + 1 KV
head per core). Per-core: project Q (4 heads), K, V from x[b]; RoPE;
causal attention in transposed layout (scores^T = [k, q], softmax
denominator via ones-column in the PV matmul); AllGather ctx^T across the
4 cores of each batch; each core computes a distinct 512-wide column slice
of the output projection.

All matmuls run as float32r (full-rate fp32, ~1.6e-4 rel err).
"""
import numpy as np

import concourse.bacc as bacc
import concourse.tile as tile
import concourse.mybir as mybir
from concourse.bass_utils import run_bass_kernel_spmd
from concourse.masks import make_identity, make_upper_triangular

f32 = mybir.dt.float32
f32r = mybir.dt.float32r
Exp = mybir.ActivationFunctionType.Exp

S = 2048          # sequence length
D = 2048          # model dim
HD = 128          # head dim
NH = 4            # query heads per core
SC = S // 512     # 512-wide s-chunks
ST = S // 128     # 128-wide s-tiles
DXO = D // 128    # contraction chunks
SCALE = HD ** -0.5
N_CORES = 8
GROUPS = [[0, 1, 2, 3], [4, 5, 6, 7]]

_CACHE = {}


def _build():
    nc = bacc.Bacc("TRN2", target_bir_lowering=False, debug=False,
                   enable_asserts=True, num_devices=N_CORES)

    x_d = nc.dram_tensor("x", [S, D], f32, kind="ExternalInput")
    wq_d = nc.dram_tensor("wq", [NH * HD, D], f32, kind="ExternalInput")
    wk_d = nc.dram_tensor("wk", [HD, D], f32, kind="ExternalInput")
    wv_d = nc.dram_tensor("wv", [HD, D], f32, kind="ExternalInput")
    wo_d = nc.dram_tensor("wo", [512, D], f32, kind="ExternalInput")
    cos_d = nc.dram_tensor("cos", [S, HD], f32, kind="ExternalInput")
    sin_d = nc.dram_tensor("sin", [S, HD], f32, kind="ExternalInput")
    out_d = nc.dram_tensor("out", [S, 512], f32, kind="ExternalOutput")

    with tile.TileContext(nc) as tc:
        with tc.tile_pool(name="const", bufs=1) as const, \
             tc.tile_pool(name="dram", bufs=1, space="DRAM") as dram:
            ident = const.tile([128, 128], f32)
            make_identity(nc, ident[:])
            tri01 = const.tile([128, 128], f32)
            make_upper_triangular(nc, tri01[:], val=1.0, diag=True)
            ones2 = const.tile([128, 2], f32)
            nc.vector.memset(ones2[:], 1.0)

            ctxT_dram = dram.tile([NH, 128, S], f32r)
            gathered = dram.tile([4 * NH, 128, S], f32r)

            with tc.tile_pool(name="persistA", bufs=1) as persistA:
                wqT = persistA.tile([128, DXO, NH * 128], f32r)  # [dx, dxo, e]
                wkT = persistA.tile([128, DXO, 128], f32r)
                wvT = persistA.tile([128, DXO, 128], f32r)
                cosT = persistA.tile([128, S], f32)              # [hd, s]
                sinTs = persistA.tile([128, S], f32)             # signed sin^T

                # ---------- weight / table transposes ----------
                with tc.tile_pool(name="wstage", bufs=1) as wstage, \
                     tc.tile_pool(name="wpsum", bufs=2, space="PSUM") as wpsum:
                    wq_sb = wstage.tile([128, NH, D], f32, tag="wq")
                    nc.sync.dma_start(
                        wq_sb[:], wq_d.ap().rearrange("(eo p) d -> p eo d", p=128))
                    for dxo in range(DXO):
                        tp = wpsum.tile([128, 512], f32, tag="wt")
                        for eo in range(NH):
                            nc.tensor.transpose(
                                tp[:, eo * 128:(eo + 1) * 128],
                                wq_sb[:, eo, dxo * 128:(dxo + 1) * 128], ident[:])
                        nc.vector.tensor_copy(wqT[:, dxo, :], tp[:])

                    for (w_in, wT) in ((wk_d, wkT), (wv_d, wvT)):
                        w_sb = wstage.tile([128, D], f32, tag="wkv")
                        nc.sync.dma_start(w_sb[:], w_in.ap())
                        for q4 in range(DXO // 4):
                            tp = wpsum.tile([128, 512], f32, tag="wt")
                            for i in range(4):
                                dxo = q4 * 4 + i
                                nc.tensor.transpose(
                                    tp[:, i * 128:(i + 1) * 128],
                                    w_sb[:, dxo * 128:(dxo + 1) * 128], ident[:])
                            nc.vector.tensor_copy(wT[:, q4 * 4:q4 * 4 + 4, :], tp[:])

                    for (t_in, tT, signed) in ((cos_d, cosT, False),
                                               (sin_d, sinTs, True)):
                        t_sb = wstage.tile([128, ST, HD], f32, tag="cs")
                        nc.sync.dma_start(
                            t_sb[:], t_in.ap().rearrange("(so p) h -> p so h", p=128))
                        for q4 in range(ST // 4):
                            tp = wpsum.tile([128, 512], f32, tag="wt")
                            for i in range(4):
                                nc.tensor.transpose(
                                    tp[:, i * 128:(i + 1) * 128],
                                    t_sb[:, q4 * 4 + i, :], ident[:])
                            sl = slice(q4 * 512, q4 * 512 + 512)
                            if signed:
                                nc.vector.tensor_scalar_mul(
                                    tT[0:64, sl], tp[0:64, :], -1.0)
                                nc.vector.tensor_copy(tT[64:128, sl], tp[64:128, :])
                            else:
                                nc.vector.tensor_copy(tT[:, sl], tp[:])

                with tc.tile_pool(name="persistB", bufs=1) as persistB:
                    qT = persistB.tile([128, NH, S], f32r)       # [hd, h, s]
                    kT = persistB.tile([128, S], f32r)
                    vaug = persistB.tile([128, ST, 132], f32r)   # [k, kt, dv|1|1]

                    # ---------- x^T + projections + RoPE ----------
                    with tc.tile_pool(name="xstage", bufs=3) as xstage, \
                         tc.tile_pool(name="xtc", bufs=1) as xtc_pool, \
                         tc.tile_pool(name="rope", bufs=3) as rope, \
                         tc.tile_pool(name="vst", bufs=2) as vst, \
                         tc.tile_pool(name="xpsum", bufs=2, space="PSUM") as xpsum, \
                         tc.tile_pool(name="ppsum", bufs=2, space="PSUM") as ppsum:
                        for sc in range(SC):
                            ssl = slice(sc * 512, sc * 512 + 512)
                            xTc = xtc_pool.tile([128, DXO, 512], f32r, tag="xTc")
                            for dxo in range(DXO):
                                xst = xstage.tile([128, 4, 128], f32, tag="xst")
                                nc.sync.dma_start(
                                    xst[:],
                                    x_d.ap()[sc * 512:(sc + 1) * 512,
                                             dxo * 128:(dxo + 1) * 128]
                                    .rearrange("(so p) d -> p so d", p=128))
                                tp = xpsum.tile([128, 512], f32, tag="xt")
                                for si in range(4):
                                    nc.tensor.transpose(
                                        tp[:, si * 128:(si + 1) * 128],
                                        xst[:, si, :], ident[:])
                                nc.vector.tensor_copy(xTc[:, dxo, :], tp[:])

                            for eo in range(NH + 1):
                                pq = ppsum.tile([128, 512], f32, tag="proj")
                                for dxo in range(DXO):
                                    if eo < NH:
                                        lhsT = wqT[:, dxo, eo * 128:(eo + 1) * 128]
                                    else:
                                        lhsT = wkT[:, dxo, :]
                                    nc.tensor.matmul(pq[:], lhsT, xTc[:, dxo, :],
                                                     start=(dxo == 0),
                                                     stop=(dxo == DXO - 1))
                                dst = qT[:, eo, ssl] if eo < NH else kT[:, ssl]
                                tmp = rope.tile([128, 512], f32, tag="ropetmp")
                                nc.vector.tensor_mul(tmp[0:64, :], pq[64:128, :],
                                                     sinTs[0:64, ssl])
                                nc.vector.tensor_mul(tmp[64:128, :], pq[0:64, :],
                                                     sinTs[64:128, ssl])
                                qcos = rope.tile([128, 512], f32, tag="ropecos")
                                nc.vector.tensor_mul(qcos[:], pq[:], cosT[:, ssl])
                                nc.vector.tensor_add(dst, qcos[:], tmp[:])

                            pv = ppsum.tile([128, 512], f32, tag="proj")
                            for dxo in range(DXO):
                                nc.tensor.matmul(pv[:], wvT[:, dxo, :],
                                                 xTc[:, dxo, :],
                                                 start=(dxo == 0),
                                                 stop=(dxo == DXO - 1))
                            vT_sb = vst.tile([128, 512], f32, tag="vT")
                            nc.vector.tensor_copy(vT_sb[:], pv[:])
                            tpv = xpsum.tile([128, 512], f32, tag="xt")
                            for si in range(4):
                                nc.tensor.transpose(
                                    tpv[:, si * 128:(si + 1) * 128],
                                    vT_sb[:, si * 128:(si + 1) * 128],
                                    ident[:])
                            for si in range(4):
                                kt = sc * 4 + si
                                nc.vector.tensor_copy(
                                    vaug[:, kt, 0:128],
                                    tpv[:, si * 128:(si + 1) * 128])
                                nc.vector.tensor_copy(vaug[:, kt, 128:130],
                                                      ones2[:])

                    with tc.tile_pool(name="persistC", bufs=1) as persistC:
                        ctxT = persistC.tile([128, NH, S], f32r)  # [dv, h, s]

                        # ---------- attention per head ----------
                        with tc.tile_pool(name="pt", bufs=18) as ptp, \
                             tc.tile_pool(name="cnat", bufs=3) as cnat, \
                             tc.tile_pool(name="small", bufs=4) as small, \
                             tc.tile_pool(name="spsum", bufs=2, space="PSUM") as spsum, \
                             tc.tile_pool(name="cpsum", bufs=3, space="PSUM") as cpsum, \
                             tc.tile_pool(name="tpsum", bufs=2, space="PSUM") as tpsum:
                            for h in range(NH):
                                for qc in range(SC):
                                    qsl = slice(qc * 512, qc * 512 + 512)
                                    nkt = 4 * qc + 4
                                    pts = []
                                    for kt in range(nkt):
                                        sp = spsum.tile([128, 512], f32,
                                                        tag="scorep")
                                        nc.tensor.matmul(
                                            sp[:],
                                            kT[:, kt * 128:(kt + 1) * 128],
                                            qT[:, h, qsl],
                                            start=True, stop=True)
                                        pt = ptp.tile([128, 512], f32r, tag="pt")
                                        nc.scalar.activation(pt[:], sp[:], Exp,
                                                             scale=SCALE)
                                        if kt >= 4 * qc:  # diagonal block
                                            c0 = kt * 128 - qc * 512
                                            nc.vector.tensor_mul(
                                                pt[:, c0:c0 + 128],
                                                pt[:, c0:c0 + 128].bitcast(f32),
                                                tri01[:])
                                        pts.append(pt)
                                    for qbl in range(4):
                                        qb = qc * 4 + qbl
                                        cp = cpsum.tile([128, 130], f32,
                                                        tag="ctxp")
                                        for kt in range(qb + 1):
                                            nc.tensor.matmul(
                                                cp[:],
                                                pts[kt][:, qbl * 128:(qbl + 1) * 128],
                                                vaug[:, kt, 0:130],
                                                start=(kt == 0), stop=(kt == qb))
                                        recip = small.tile([128, 1], f32,
                                                           tag="recip")
                                        nc.vector.reciprocal(recip[:],
                                                             cp[:, 128:129])
                                        cn = cnat.tile([128, 128], f32, tag="cn")
                                        nc.vector.tensor_scalar_mul(
                                            cn[:], cp[:, 0:128], recip[:])
                                        tp2 = tpsum.tile([128, 128], f32,
                                                         tag="tctx")
                                        nc.tensor.transpose(tp2[:], cn[:],
                                                            ident[:])
                                        nc.vector.tensor_copy(
                                            ctxT[:, h, qb * 128:(qb + 1) * 128],
                                            tp2[:])

                        # ---------- AllGather ctx^T ----------
                        nc.sync.dma_start(
                            ctxT_dram[:].rearrange("h p s -> p h s"), ctxT[:])
                        nc.gpsimd.collective_compute(
                            "AllGather", mybir.AluOpType.bypass,
                            replica_groups=GROUPS,
                            ins=[ctxT_dram[:]], outs=[gathered[:]])

            # ---------- output projection ----------
            with tc.tile_pool(name="woT", bufs=1) as woTp, \
                 tc.tile_pool(name="wopsum", bufs=2, space="PSUM") as wopsum:
                woT = woTp.tile([128, DXO, 512], f32r)   # [e, ec, d]
                with tc.tile_pool(name="wostage", bufs=1) as wostage:
                    wo_sb = wostage.tile([128, 4, D], f32)
                    nc.sync.dma_start(
                        wo_sb[:], wo_d.ap().rearrange("(do p) e -> p do e", p=128))
                    for ec in range(DXO):
                        tp = wopsum.tile([128, 512], f32, tag="wot")
                        for do in range(4):
                            nc.tensor.transpose(
                                tp[:, do * 128:(do + 1) * 128],
                                wo_sb[:, do, ec * 128:(ec + 1) * 128], ident[:])
                        nc.vector.tensor_copy(woT[:, ec, :], tp[:])

                with tc.tile_pool(name="ctxF", bufs=1) as ctxFp, \
                     tc.tile_pool(name="osb", bufs=3) as osb, \
                     tc.tile_pool(name="opsum", bufs=2, space="PSUM") as opsum:
                    ctxF = ctxFp.tile([128, 4 * NH, S], f32r)
                    for ec in range(4 * NH):
                        nc.sync.dma_start(ctxF[:, ec, :], gathered[ec])

                    for st in range(ST):
                        op = opsum.tile([128, 512], f32, tag="op")
                        for ec in range(4 * NH):
                            nc.tensor.matmul(
                                op[:], ctxF[:, ec, st * 128:(st + 1) * 128],
                                woT[:, ec, :],
                                start=(ec == 0), stop=(ec == 4 * NH - 1))
                        o_sb = osb.tile([128, 512], f32, tag="osb")
                        nc.vector.tensor_copy(o_sb[:], op[:])
                        nc.sync.dma_start(
                            out_d.ap()[st * 128:(st + 1) * 128, :], o_sb[:])

    nc.compile()
    return nc


def kernel(x, mask, cos, sin, Wq, Wk, Wv, Wo):
    x = np.ascontiguousarray(np.asarray(x, dtype=np.float32))
    cos = np.ascontiguousarray(np.asarray(cos, dtype=np.float32))
    sin = np.ascontiguousarray(np.asarray(sin, dtype=np.float32))
    Wq = np.ascontiguousarray(np.asarray(Wq, dtype=np.float32))
    Wk = np.ascontiguousarray(np.asarray(Wk, dtype=np.float32))
    Wv = np.ascontiguousarray(np.asarray(Wv, dtype=np.float32))
    Wo = np.ascontiguousarray(np.asarray(Wo, dtype=np.float32))

    if "nc" not in _CACHE:
        _CACHE["nc"] = _build()
    nc = _CACHE["nc"]

    in_maps = []
    for c in range(N_CORES):
        b, g = c // 4, c % 4
        in_maps.append({
            "x": x[b],
            "wq": np.ascontiguousarray(Wq[g * 512:(g + 1) * 512]),
            "wk": np.ascontiguousarray(Wk[g * 128:(g + 1) * 128]),
            "wv": np.ascontiguousarray(Wv[g * 128:(g + 1) * 128]),
            "wo": np.ascontiguousarray(Wo[g * 512:(g + 1) * 512]),
            "cos": cos,
            "sin": sin,
        })

    res = run_bass_kernel_spmd(nc, in_maps, list(range(N_CORES)))

    B = x.shape[0]
    out = np.empty((B, S, D), dtype=np.float32)
    for c in range(N_CORES):
        b, g = c // 4, c % 4
        out[b][:, g * 512:(g + 1) * 512] = res.results[c]["out"]
    return out
